# revision 35
# baseline (speedup 1.0000x reference)
"""Compressed Interaction Network (CIN) forward on 8 Trainium2 NeuronCores.

Math (per batch item, m=32 fields, d=64 embed, H=256 hidden):
    x0 = x[i]                          # (m, d)
    h  = x0
    layer l in 0..2:
        z = outer(x0, h) over d        # (m*n, d), z[(a,b),:] = x0[a,:]*h[b,:]
        y = relu(W_l^T z + b_l)        # (H, d)
        xcur, h = split_half(y) (layers 0,1); xcur = h = y (layer 2)
    f = concat(xcur_0, xcur_1, xcur_2) # (512, d)
    out[i] = sum_d(f) @ fc_W + fc_b    # scalar

Mapping: batch 1024 -> 8 cores x 128 items, 16 groups of 8 items per core.

v4 design:
 - Layer 2 runs in fp8-e4m3 DoubleRow mode: W2 host-quantized to fp8 scaled
   x16 (undone via the ACT scale), z2 in fp8.  DR matmuls contract two 128-k
   chunks per instruction at ~222ns (HW-measured ~2x fp16).  L0/L1 stay
   fp16: one fp8 layer costs ~1.15e-2 rel err (gate 2e-2); more would breach.
 - fp8 z2 production was the hard part.  HW measurements: DVE tensor ops
   drop to 1x mode with any 1-byte operand (2.28us/2048-el quad), the Pool
   engine contends with DVE (both degrade ~1.6-2.7x when streaming
   concurrently -- v3 learned this the hard way), but ACT converts a
   2048-el quad fp16->fp8 in 2.0us flat and is immune to the storm.  So:
   DVE builds z2 quads in cheap fp16 2x mode (1.21us each), ACT converts
   quads 0-6 to fp8, and quad 7 is built fp8-direct on DVE so its convert
   never races the L2 consumption deadline.  Pool does nothing.
 - z layout m-major ([128 n, m, item, d]) so DR pairs adjacent m-chunks;
   host ships xm (m-major x) for contiguous bg DMA, and ships the layer-0
   symmetric-pair products z0 directly (the host already folds W0).
 - PE iteration order [L1(i)][L0(i+1)][L2(i-1)]: L2 skewed one group so the
   h2(i) -> z2(i) build+convert overlaps a full iteration of PE work.
 - L2(g) consumes quads in order [q0..q5, q7, q6] (PSUM accumulation is
   order-free): q6's convert lands ~21.3us into the next iter, so it goes
   last; fp8-direct q7 is ready long before.
 - DVE FIFO per iter: [reds][z2f16 q0-3][z1 c0,c1][q4][c2,c3][q5][c4,c5]
   [q6][c6,c7][z2f8 q7] -- verified against per-chunk consumption deadlines.
 - ACT FIFO per iter: [act_l1(i)][cv(i-1,q5)][act_l0(i+1)][cv(i-1,q6)]
   [act_l2(i-1)][cv(i,q0-4)].
"""

import numpy as np
import ml_dtypes

import concourse.bass as bass
import concourse.tile as tile
from concourse import mybir
from concourse.bass_utils import run_bass_kernel_spmd

N_CORES = 8
B_TOTAL = 1024
B_CORE = B_TOTAL // N_CORES  # 128
M = 32  # num fields
D = 64  # embed dim
H = 256  # conv output channels
GROUP = 8  # items per group (512 moving columns)
N_GROUPS = B_CORE // GROUP  # 16
NP0 = (M * (M + 1)) // 2  # 528 unique symmetric pairs in layer 0
C0 = (NP0 + 127) // 128  # 5 k-chunks of 128
XQF = C0 * GROUP * D  # 2560 z0 elems per partition per group

W8SCALE = 16.0  # fp8 W2 pre-scale, undone in the L2 ACT

F16 = mybir.dt.float16
F32 = mybir.dt.float32
F8 = mybir.dt.float8e4
DR = mybir.MatmulPerfMode.DoubleRow
RELU = mybir.ActivationFunctionType.Relu
IDENT = mybir.ActivationFunctionType.Identity
COPY = mybir.ActivationFunctionType.Copy
AX_X = mybir.AxisListType.X

E4NP = ml_dtypes.float8_e4m3

# L2 DR pair consumption order (pair t reads quad t//2); q6 last (late cv).
L2_T_ORDER = [0, 1, 2, 3, 4, 5, 6, 7, 8, 9, 10, 11, 14, 15, 12, 13]


def build():
    nc = bass.Bass()
    xm = nc.declare_dram_parameter("xm", [N_GROUPS, M, GROUP, D], F16, isOutput=False)
    z0h = nc.declare_dram_parameter("z0h", [N_GROUPS, 128, XQF], F16, isOutput=False)
    w0s = nc.declare_dram_parameter("w0s", [128, C0, H], F16, isOutput=False)
    w1 = nc.declare_dram_parameter("w1", [128, 32, H], F16, isOutput=False)
    w2 = nc.declare_dram_parameter("w2", [128, 32, H], F8, isOutput=False)
    bia = nc.declare_dram_parameter("bia", [128, 3, 2], F32, isOutput=False)
    fcw = nc.declare_dram_parameter("fcw", [128, 4], F32, isOutput=False)
    fcb = nc.declare_dram_parameter("fcb", [1, 1], F32, isOutput=False)
    out = nc.declare_dram_parameter("out", [B_CORE, 1], F32, isOutput=True)

    with tile.TileContext(nc) as tc:
        with (
            tc.tile_pool(name="consts", bufs=1) as consts,
            tc.tile_pool(name="bgpool", bufs=4) as bgpool,      # 4 x 16KB halves
            tc.tile_pool(name="z0pool", bufs=2) as z0pool,      # 2 x 5KB
            tc.tile_pool(name="z1pool", bufs=8) as z1pool,      # 8 x 4KB
            tc.tile_pool(name="zspool", bufs=6) as zspool,      # 6 x 4KB f16 scratch
            tc.tile_pool(name="z2pool", bufs=16) as z2pool,     # 16 x 2KB f8 quads
            tc.tile_pool(name="hpool", bufs=3) as hpool,
            tc.tile_pool(name="rypool", bufs=7) as rypool,
            tc.tile_pool(name="spool", bufs=1) as spool,
            tc.tile_pool(name="ppool", bufs=6, space="PSUM") as ppool,
            tc.tile_pool(name="fcp", bufs=1, space="PSUM") as fcp,
        ):
            w0s_sb = consts.tile([128, C0, H], F16, tag="w0s")
            w1_sb = [
                consts.tile([128, 16, H], F16, tag=f"w1{h}", name=f"w1{h}")
                for h in range(2)
            ]
            w2_sb = consts.tile([128, 32, H], F8, tag="w2")
            wscr_sb = consts.tile([128, 2, 256], F16, tag="wscr")
            bia_sb = consts.tile([128, 3, 2], F32, tag="bia")
            fcw_sb = consts.tile([128, 4], F32, tag="fcw")
            fcb_sb = consts.tile([1, 1], F32, tag="fcb")

            s_sb = spool.tile([128, 4, B_CORE], F32, tag="s")

            bg_t = {}    # (g, half) -> [128, 16, GROUP, D] f16
            z0_t = {}
            z1_t = {}    # (g, c) -> [128, 4, GROUP, D] f16, c in 0..7
            zs_t = {}    # (g, q) -> f16 scratch quad awaiting convert
            z2_t = {}    # (g, q) -> [128, 4, GROUP, D] f8
            h1_t, h2_t = {}, {}
            ry_t = {}
            ps01_t, ps12_t = {}, {}

            def dma_z0(g, eng=None):
                t = z0pool.tile([128, C0, GROUP, D], F16, tag="z0")
                ap = bass.AP(
                    tensor=z0h,
                    offset=g * 128 * XQF,
                    ap=[[XQF, 128], [1, XQF]],
                )
                (eng or nc.sync).dma_start(t[:], ap)
                z0_t[g] = t

            def dma_bg_half(g, mh, eng=None):
                t = bgpool.tile([128, 16, GROUP, D], F16, tag="B", name="B")
                src = bass.AP(
                    tensor=xm,
                    offset=g * M * GROUP * D + mh * 16 * GROUP * D,
                    ap=[[0, 128], [1, 16 * GROUP * D]],
                )
                (eng or nc.sync).dma_start(t[:], src)
                bg_t[(g, mh)] = t

            def dma_bg(g):
                dma_bg_half(g, 0)
                dma_bg_half(g, 1)

            def mm_l0(g):
                ps = [
                    ppool.tile([128, GROUP * D], F32, tag="ps", name="ps0")
                    for _ in range(2)
                ]
                for c in range(C0):
                    for oc in range(2):
                        nc.tensor.matmul(
                            ps[oc][:],
                            w0s_sb[:, c, oc * 128 : (oc + 1) * 128],
                            z0_t[g][:, c, :, :],
                            start=(c == 0),
                            stop=(c == C0 - 1),
                        )
                ps01_t[g] = ps
                del z0_t[g]

            def act_l0(g):
                ps = ps01_t[g]
                h = hpool.tile([128, GROUP, D], F16, tag="h1")
                nc.scalar.activation(h[:], ps[1][:], RELU, bias=bia_sb[:, 0, 1:2])
                h1_t[g] = h
                r = rypool.tile([128, GROUP, D], F16, tag="ry")
                nc.scalar.activation(r[:], ps[0][:], RELU, bias=bia_sb[:, 0, 0:1])
                ry_t[(g, 0)] = r
                del ps01_t[g]

            def build_z1(g, cs, last=False):
                """z1 chunks (4 m wide each) for group g, m-major fp16, DVE."""
                h = h1_t[g]
                for c in cs:
                    zt = z1pool.tile([128, 4, GROUP, D], F16, tag="z1")
                    nc.vector.tensor_mul(
                        zt[:],
                        h[:, None, :, :].to_broadcast((128, 4, GROUP, D)),
                        bg_t[(g, c // 4)][:, (c % 4) * 4 : (c % 4) * 4 + 4, :, :],
                    )
                    z1_t[(g, c)] = zt
                if last:
                    del h1_t[g]

            def mm_l12_1(g):
                ps = [
                    ppool.tile([128, GROUP * D], F32, tag="ps", name="ps1")
                    for _ in range(2)
                ]
                for m in range(32):
                    wt, mi = w1_sb[m // 16], m % 16
                    zt = z1_t[(g, m // 4)]
                    for oc in range(2):
                        nc.tensor.matmul(
                            ps[oc][:],
                            wt[:, mi, oc * 128 : (oc + 1) * 128],
                            zt[:, m % 4, :, :],
                            start=(m == 0),
                            stop=(m == 31),
                        )
                    if m % 4 == 3:
                        del z1_t[(g, m // 4)]
                ps12_t[g] = ps

            def act_l1(g):
                ps = ps12_t[g]
                h = hpool.tile([128, GROUP, D], F16, tag="h2")
                nc.scalar.activation(h[:], ps[1][:], RELU, bias=bia_sb[:, 1, 1:2])
                h2_t[g] = h
                r = rypool.tile([128, GROUP, D], F16, tag="ry")
                nc.scalar.activation(r[:], ps[0][:], RELU, bias=bia_sb[:, 1, 0:1])
                ry_t[(g, 1)] = r
                del ps12_t[g]

            def build_z2f16(g, qs):
                """fp16 z2 scratch quads on DVE (2x mode), converted by ACT."""
                h = h2_t[g]
                for q in qs:
                    zt = zspool.tile([128, 4, GROUP, D], F16, tag="zs")
                    nc.vector.tensor_mul(
                        zt[:],
                        h[:, None, :, :].to_broadcast((128, 4, GROUP, D)),
                        bg_t[(g, q // 4)][:, (q % 4) * 4 : (q % 4) * 4 + 4, :, :],
                    )
                    zs_t[(g, q)] = zt

            def build_z2f8(g, qs, final=False):
                """fp8 z2 quads built directly on DVE (1x mode)."""
                h = h2_t[g]
                for q in qs:
                    zt = z2pool.tile([128, 4, GROUP, D], F8, tag="z2")
                    nc.vector.tensor_mul(
                        zt[:],
                        h[:, None, :, :].to_broadcast((128, 4, GROUP, D)),
                        bg_t[(g, q // 4)][:, (q % 4) * 4 : (q % 4) * 4 + 4, :, :],
                    )
                    z2_t[(g, q)] = zt
                if final:
                    del h2_t[g]

            def cv_z2(g, qs, final=False):
                """ACT converts f16 scratch quads to fp8."""
                for q in qs:
                    zt = z2pool.tile([128, 4, GROUP, D], F8, tag="z2")
                    nc.scalar.activation(zt[:], zs_t[(g, q)][:], COPY)
                    z2_t[(g, q)] = zt
                    del zs_t[(g, q)]
                if final:
                    del h2_t[g]

            def mm_l2(g, t_order):
                ps = [
                    ppool.tile([128, GROUP * D], F32, tag="ps", name="ps2")
                    for _ in range(2)
                ]
                for n, t in enumerate(t_order):
                    zt = z2_t[(g, t // 2)]
                    mv = zt[:, (t % 2) * 2 : (t % 2) * 2 + 2, :, :]
                    for oc in range(2):
                        nc.tensor.matmul(
                            ps[oc][:],
                            w2_sb[:, 2 * t : 2 * t + 2, oc * 128 : (oc + 1) * 128],
                            mv,
                            start=(n == 0),
                            stop=(n == 15),
                            perf_mode=DR,
                        )
                for q in range(8):
                    del z2_t[(g, q)]
                ps12_t[g] = ps

            def act_l2(g):
                ps = ps12_t[g]
                for oc in range(2):
                    r = rypool.tile([128, GROUP, D], F16, tag="ry")
                    nc.scalar.activation(
                        r[:],
                        ps[oc][:],
                        RELU,
                        bias=bia_sb[:, 2, oc : oc + 1],
                        scale=1.0 / W8SCALE,
                    )
                    ry_t[(g, 2 + oc)] = r
                del ps12_t[g]

            def red(g, chunk):
                i0 = g * GROUP
                nc.vector.reduce_sum(
                    s_sb[:, chunk, i0 : i0 + GROUP],
                    ry_t[(g, chunk)][:],
                    axis=AX_X,
                )
                del ry_t[(g, chunk)]

            def pe_filler(n, stat, mov):
                wps = [
                    ppool.tile([128, GROUP * D], F32, tag="ps", name="warm")
                    for _ in range(2)
                ]
                for wi in range(n):
                    nc.tensor.matmul(
                        wps[wi % 2][:],
                        stat,
                        mov,
                        start=(wi < 2),
                        stop=(wi >= n - 2),
                    )

            # ---------------- prologue ----------------
            nc.gpsimd.memset(wscr_sb[:], 0.0)
            dma_z0(0)
            nc.sync.dma_start(w0s_sb[:], w0s[:, :, :])
            nc.sync.dma_start(bia_sb[:], bia[:])
            dma_bg_half(0, 0)
            nc.sync.dma_start(w1_sb[0][:], w1[:, 0:16, :])
            dma_bg_half(0, 1)
            nc.sync.dma_start(w1_sb[1][:], w1[:, 16:32, :])
            nc.sync.dma_start(w2_sb[:], w2[:, :, :])
            dma_z0(1)
            dma_bg(1)
            nc.sync.dma_start(fcw_sb[:], fcw[:])
            nc.sync.dma_start(fcb_sb[:], fcb[:])

            pe_filler(14, wscr_sb[:, 0, 0:128], wscr_sb[:, :, :])
            mm_l0(0)
            pe_filler(12, wscr_sb[:, 0, 0:128], wscr_sb[:, :, :])
            act_l0(0)
            build_z1(0, range(8), last=True)

            # ---------------- steady-state pipeline ----------------
            for i in range(N_GROUPS):
                last = i == N_GROUPS - 1
                # --- PE slot 1: L1(i) ---
                mm_l12_1(i)
                act_l1(i)
                if i >= 1 and i - 1 != N_GROUPS - 1:
                    cv_z2(i - 1, (5,))
                if i >= 2:
                    red(i - 2, 2)
                    red(i - 2, 3)
                if i >= 1:
                    red(i - 1, 1)
                red(i, 0)
                if i + 2 < N_GROUPS:
                    dma_z0(i + 2)
                # --- PE slot 2: L0(i+1); DVE: z2f16/z1 interleave ---
                if i == 0:
                    # iter 0 only: z1(1) chunks lead the DVE FIFO (L1(1)
                    # otherwise stalls behind z2(0) quads whose deadline is
                    # 1.5 iterations away)
                    mm_l0(i + 1)
                    act_l0(i + 1)
                    build_z1(i + 1, (0, 1))
                    build_z2f16(i, (0,))
                    build_z1(i + 1, (2, 3))
                    build_z2f16(i, (1,))
                    build_z1(i + 1, (4, 5))
                    build_z2f16(i, (2,))
                    build_z1(i + 1, (6, 7), last=True)
                    build_z2f16(i, (3, 4, 5, 6))
                    build_z2f8(i, (7,), final=True)
                elif not last:
                    mm_l0(i + 1)
                    act_l0(i + 1)
                    cv_z2(i - 1, (6,))
                    build_z2f16(i, (0, 1, 2, 3))
                    build_z1(i + 1, (0, 1))
                    build_z2f16(i, (4,))
                    build_z1(i + 1, (2, 3))
                    build_z2f16(i, (5,))
                    build_z1(i + 1, (4, 5))
                    build_z2f16(i, (6,))
                    build_z1(i + 1, (6, 7), last=True)
                    build_z2f8(i, (7,), final=True)
                else:
                    # group 15: no z1(16); build all z2 direct-fp8 on DVE
                    pe_filler(10, w0s_sb[:, 0, 0:128], w0s_sb[:, 0:2, :])
                    cv_z2(i - 1, (6,))
                    build_z2f8(i, range(8), final=True)
                if i + 2 < N_GROUPS:
                    dma_bg(i + 2)
                # --- PE slot 3: L2(i-1) ---
                if i == 0:
                    pe_filler(10, w0s_sb[:, 0, 0:128], w0s_sb[:, 0:2, :])
                if i >= 1:
                    mm_l2(i - 1, L2_T_ORDER)
                    act_l2(i - 1)
                if not last:
                    cv_z2(i, (0, 1, 2, 3, 4))

            # ---------------- epilogue ----------------
            g = N_GROUPS - 1
            red(g - 1, 2)
            red(g - 1, 3)
            red(g, 1)
            mm_l2(g, range(16))
            # FC part A (items 0-119, groups 0-14) overlaps L2(15)'s tail
            fc_ps = fcp.tile([1, B_CORE], F32, tag="fc")
            for c in range(4):
                nc.tensor.matmul(
                    fc_ps[:, 0:120],
                    fcw_sb[:, c : c + 1],
                    s_sb[:, c, 0:120],
                    start=(c == 0),
                    stop=(c == 3),
                )
            act_l2(g)
            red(g, 2)
            red(g, 3)
            for c in range(4):
                nc.tensor.matmul(
                    fc_ps[:, 120:128],
                    fcw_sb[:, c : c + 1],
                    s_sb[:, c, 120:128],
                    start=(c == 0),
                    stop=(c == 3),
                )
            osb = consts.tile([1, B_CORE], F32, tag="osb")
            nc.scalar.activation(osb[:], fc_ps[:], IDENT, bias=fcb_sb[0:1, 0:1])
            nc.sync.dma_start(out[:], osb[:])

    _legalize_waits(nc)
    return nc


def _legalize_waits(nc, max_waits=1):
    """walrus codegen allows at most 2 semaphore waits per instruction; spill
    the excess onto NoOps injected just before the offender on the same
    engine (same-engine FIFO makes this ordering-equivalent)."""
    for bb in nc.main_func.blocks:
        insts = bb.instructions
        new_list = []
        changed = False
        for ins in insts:
            si = ins.sync_info
            if si is not None and si.on_wait and len(si.on_wait) > max_waits:
                waits = list(si.on_wait)
                extra, keep = waits[:-max_waits], waits[-max_waits:]
                k = 0
                while k < len(extra):
                    chunk = extra[k : k + max_waits]
                    nop = mybir.InstNoOp(name=f"{ins.name}-w{k}", ins=[], outs=[])
                    nop.engine = ins.engine
                    nop.sync_info = mybir.SyncInfo(on_wait=chunk, on_update=[])
                    new_list.append(nop)
                    k += max_waits
                ins.sync_info = mybir.SyncInfo(
                    on_wait=keep,
                    on_update=list(si.on_update) if si.on_update else [],
                )
                changed = True
            new_list.append(ins)
        if changed:
            if hasattr(bb, "set_instructions"):
                bb.set_instructions(new_list)
            else:
                insts.clear()
                insts.extend(new_list)
                if len(bb.instructions) != len(new_list):
                    bb.instructions = new_list


def _sym_maps():
    a, b = np.triu_indices(M)
    pad = C0 * 128 - NP0
    amap = np.concatenate([a, np.zeros(pad, np.int64)])
    bmap = np.concatenate([b, np.zeros(pad, np.int64)])
    return amap, bmap


def prep_inputs(x, W0, b0, W1, b1, W2, b2, fc_W, fc_b):
    """Host-side reshape/cast into the per-core input maps."""
    xh = np.ascontiguousarray(x.astype(np.float16))
    xmf = np.ascontiguousarray(
        xh.reshape(B_TOTAL // GROUP, GROUP, M, D).transpose(0, 2, 1, 3)
    )
    amap, bmap = _sym_maps()
    idx_a = amap.reshape(C0, 128).T
    idx_b = bmap.reshape(C0, 128).T

    def _hw(idx):
        g = xh[:, idx, :]  # (B_TOTAL, 128, C0, D)
        g = g.reshape(B_TOTAL // GROUP, GROUP, 128, C0, D)
        g = g.transpose(0, 2, 3, 1, 4)
        return g.reshape(B_TOTAL // GROUP, 128, C0 * GROUP * D)

    # layer-0 symmetric-pair products, fp16 (pad rows hold garbage products
    # but their folded W0 rows are zero)
    z0 = np.ascontiguousarray(_hw(idx_a) * _hw(idx_b))

    W0r = np.asarray(W0, np.float32).reshape(M, M, H)
    Wsym = W0r[amap[:NP0], bmap[:NP0]] + np.where(
        (amap[:NP0] != bmap[:NP0])[:, None], W0r[bmap[:NP0], amap[:NP0]], 0.0
    )
    Wpad = np.zeros((C0 * 128, H), np.float32)
    Wpad[:NP0] = Wsym
    w0s = np.ascontiguousarray(
        Wpad.astype(np.float16).reshape(C0, 128, H).transpose(1, 0, 2)
    )
    w1 = np.ascontiguousarray(
        W1.astype(np.float16).reshape(32, 128, H).transpose(1, 0, 2)
    )
    w2 = np.ascontiguousarray(
        np.clip(np.asarray(W2, np.float32) * W8SCALE, -240, 240)
        .astype(E4NP)
        .reshape(32, 128, H)
        .transpose(1, 0, 2)
    )
    bia = np.ascontiguousarray(
        np.stack([b0, b1, b2]).reshape(3, 2, 128).transpose(2, 0, 1).astype(np.float32)
    )
    fcw = np.ascontiguousarray(fc_W.reshape(4, 128).T.astype(np.float32))
    fcb = np.ascontiguousarray(fc_b.reshape(1, 1).astype(np.float32))
    shared = {"w0s": w0s, "w1": w1, "w2": w2, "bia": bia, "fcw": fcw, "fcb": fcb}
    return [
        {
            "xm": xmf[i * N_GROUPS : (i + 1) * N_GROUPS],
            "z0h": z0[i * N_GROUPS : (i + 1) * N_GROUPS],
            **shared,
        }
        for i in range(N_CORES)
    ]


_NC = None


def _get_nc():
    global _NC
    if _NC is None:
        _NC = build()
    return _NC


def kernel(**inputs):
    in_maps = prep_inputs(**inputs)
    res = run_bass_kernel_spmd(_get_nc(), in_maps, list(range(N_CORES)))
    return np.ascontiguousarray(
        np.concatenate([r["out"] for r in res.results], axis=0).astype(np.float32)
    )
